# revision 6
# baseline (speedup 1.0000x reference)
"""Trainium2 Bass kernel for nn_ModelConTT_46016279609475 (TT interpolation).

y[b] = v0[b]^T V1[b] V2[b] v3[b], where v_i are linearly-interpolated slices
of tiny TT cores at per-point grid coordinates derived from x[b, :].

Strategy (per NeuronCore, data-parallel over B):
  * Host precomputes two joint corner-packed tables (pure functions of the
    ~1MB cores, so no on-device table build or DRAM writeback):
      G[n0, n1, k] = sum_c core0[n0, c] * core1[c, n1, k]        (u-side)
      H[n2, n3, k] = sum_c core2[k, n2, c] * core3[c, n3]        (v-side)
    packed bf16 rows T[(a*128+b)] = [16 k x 4 corners] + 64 pad = 256B
    (dma_gather's minimum element), stacked G then H in ghd[32768, 128].
  * Per chunk ONE dma_gather fetches both sides for 4096 points
    (num_idxs=8192): index position i = (s*JC + j)*128 + p lands entry at
    dst[p, s*JC + j, :], so the output is directly [p, side, j, k, c].
  * Index lists are built on-device in dma_gather's wrapped layout
    (idx i at [i%16, i//16], replicated to rows 16-31 for queue 0's core
    pair) from a host-rearranged second copy of x (xq2), processed in
    16-row bands at partition bases 0/32/64/96 like the combine's x copy.
    H-side entries get +16384 by adding 128.0 to fl_d0 before the *128.
  * Combine on DVE in bf16 (2x mode): m = g * W (corner weights bcast
    over k), pairwise corner adds, u_G * u_H, reduce over k.
  * Exact-floor trick (f32-safe): t = (xc + 2^23) - 2^23 rounds to
    nearest; g = (t > xc); floor = t - g; frac = xc - floor is exact.

Batch mapping per core: shard b of size 32768; point i lives at
partition i%128, free col i//128 (y_pm[p, j] = y[j*128 + p]).
"""

import numpy as np
import ml_dtypes

import concourse.bass as bass
import concourse.bacc as bacc
import concourse.mybir as mybir
import concourse.tile as tile
from concourse import library_config
from concourse.bass_utils import run_bass_kernel_spmd

F32 = mybir.dt.float32
BF16 = mybir.dt.bfloat16
I16 = mybir.dt.int16
OP = mybir.AluOpType
AF = mybir.ActivationFunctionType

NCORES = 8
B = 262144
BS = B // NCORES          # 32768 points per core
P = 128                   # partitions
J = BS // P               # 256 free cols per partition
NCH = 8                   # pipeline chunks
JC = J // NCH             # 32 cols per chunk per side
NI = 2 * JC * P           # 8192 gathered entries per chunk (both sides)
LCC = NI // 16            # 512 idx-list cols per chunk
LT = LCC * NCH            # 4096 idx-list cols total
LB = LT // 4              # 1024 idx-list cols per 16-row band
N = 128                   # mode size
R = 16                    # TT rank
TE = N * N                # entries per table
EV = 64                   # useful values per entry: 16 k x 4 corners
ES = 128                  # stored row: EV values + pad to 256B
MAGIC = float(2 ** 23)
SCALE = (N - 1) / 2.0     # 63.5

_CACHED = None
DEBUG_TILES = {}


def _build_nc():
    nc = bacc.Bacc("TRN2")

    x_pm = nc.dram_tensor("x_pm", [P, J, 4], F32, kind="ExternalInput")
    xq2 = nc.dram_tensor("xq2", [64, LB, 2], F32, kind="ExternalInput")
    ghd = nc.dram_tensor("ghd", [2 * TE, ES], BF16, kind="ExternalInput")
    y_pm = nc.dram_tensor("y_pm", [P, J], F32, kind="ExternalOutput")

    with tile.TileContext(nc) as tc:
        with tc.tile_pool(name="per", bufs=1) as pe:
            nc.gpsimd.load_library(library_config.mlp)

            # idx list in dma_gather wrapped layout; rows 32+ only feed the
            # bounds check, memset once on Pool.
            LS = pe.tile([P, LT], I16)
            nc.gpsimd.memset(LS[:], 0)

            # ---------------- weights path (x in [p, j, d] layout) --------
            x_s = pe.tile([P, J * 4], F32)
            nc.sync.dma_start(x_s[:], x_pm[:].rearrange("p a b -> p (a b)"))
            xc = pe.tile([P, J * 4], F32)
            nc.scalar.activation(xc[:], x_s[:], AF.Copy, bias=SCALE, scale=SCALE)
            t1 = pe.tile([P, J * 4], F32)
            nc.scalar.activation(t1[:], xc[:], AF.Copy, bias=MAGIC, scale=1.0)
            gt = pe.tile([P, J * 4], F32)
            nc.vector.scalar_tensor_tensor(
                gt[:], t1[:], -MAGIC, xc[:], OP.add, OP.is_gt
            )
            fl = pe.tile([P, J * 4], F32)
            nc.vector.scalar_tensor_tensor(
                fl[:], t1[:], -MAGIC, gt[:], OP.add, OP.subtract
            )
            wv = pe.tile([P, J * 4], F32)
            nc.vector.tensor_tensor(wv[:], xc[:], fl[:], OP.subtract)
            av = pe.tile([P, J * 4], F32)
            nc.vector.tensor_scalar(av[:], wv[:], -1.0, 1.0, OP.mult, OP.add)

            wvv = wv[:].rearrange("p (j d) -> p j d", d=4)
            avv = av[:].rearrange("p (j d) -> p j d", d=4)

            # corner weights [P, 2, J, 4] bf16; corner c=(dhi,dlo):
            #   c0=a*a, c1=a*w, c2=w*a, c3=w*w over dims (0,1) / (2,3)
            WGH = pe.tile([P, 2, J, 4], BF16)
            for s, (d0, d1) in enumerate(((0, 1), (2, 3))):
                for c, (h0, h1) in enumerate(((avv, avv), (avv, wvv),
                                              (wvv, avv), (wvv, wvv))):
                    nc.vector.tensor_tensor(
                        WGH[:, s, :, c], h0[:, :, d0], h1[:, :, d1], OP.mult
                    )

            # ---------------- idx path (xq2 in wrapped band layout) -------
            xq2_s = pe.tile([112, LB * 2], F32)
            xq2v = xq2[:].rearrange("p a b -> p (a b)")
            for b in range(4):
                nc.sync.dma_start(
                    xq2_s[32 * b : 32 * b + 16, :], xq2v[16 * b : 16 * b + 16, :]
                )
            xc2 = pe.tile([112, LB * 2], F32)
            t12 = pe.tile([112, LB * 2], F32)
            g2 = pe.tile([112, LB * 2], F32)
            fl2 = pe.tile([112, LB * 2], F32)

            ysb = pe.tile([P, J], BF16)

            with (
                tc.tile_pool(name="gbuf", bufs=3) as gb,
                tc.tile_pool(name="cbuf", bufs=2) as cb,
            ):
                for ch in range(NCH):
                    # ---- per-chunk idx chain on this chunk's band rows ----
                    q = 32 * (ch // 2)              # partition base of band
                    c0 = LCC * ch - LB * (ch // 2)  # col offset inside band
                    fs = slice(2 * c0, 2 * (c0 + LCC))  # f32 value cols
                    rows = slice(q, q + 16)
                    veng = nc.vector
                    nc.scalar.activation(
                        xc2[rows, fs], xq2_s[rows, fs], AF.Copy,
                        bias=SCALE, scale=SCALE,
                    )
                    nc.scalar.activation(
                        t12[rows, fs], xc2[rows, fs], AF.Copy,
                        bias=MAGIC, scale=1.0,
                    )
                    veng.scalar_tensor_tensor(
                        g2[rows, fs], t12[rows, fs], -MAGIC, xc2[rows, fs],
                        OP.add, OP.is_gt,
                    )
                    veng.scalar_tensor_tensor(
                        fl2[rows, fs], t12[rows, fs], -MAGIC, g2[rows, fs],
                        OP.add, OP.subtract,
                    )
                    f2v = fl2[rows, fs].rearrange("p (c d) -> p c d", d=2)
                    # H-side (cols LCC/2..LCC of the chunk): fl_d0 += 128
                    # so idx = (fl_d0+128)*128 + fl_d1 lands in H rows.
                    nc.vector.tensor_scalar(
                        f2v[:, LCC // 2 :, 0], f2v[:, LCC // 2 :, 0],
                        1.0, 128.0, OP.mult, OP.add,
                    )
                    nc.vector.scalar_tensor_tensor(
                        LS[rows, LCC * ch : LCC * ch + LCC],
                        f2v[:, :, 0], 128.0, f2v[:, :, 1], OP.mult, OP.add,
                    )
                    if q != 0:
                        nc.sync.dma_start(
                            LS[0:16, LCC * ch : LCC * ch + LCC],
                            LS[rows, LCC * ch : LCC * ch + LCC],
                        )
                    nc.sync.dma_start(
                        LS[16:32, LCC * ch : LCC * ch + LCC],
                        LS[0:16, LCC * ch : LCC * ch + LCC],
                    )

                    # ---- gather both sides for this chunk ----
                    gth = gb.tile([P, 2 * JC, ES], BF16, tag="gth")
                    nc.gpsimd.dma_gather(
                        gth[:],
                        ghd[:],
                        LS[:, LCC * ch : LCC * ch + LCC],
                        NI,
                        NI,
                        ES,
                        queue_num=0,
                        single_packet=False,
                    )

                    # ---- combine (bf16 DVE) ----
                    m = cb.tile([P, 2, JC, R, 4], BF16, tag="m")
                    t2 = cb.tile([P, 2, JC, R, 2], BF16, tag="t2")
                    u = cb.tile([P, 2, JC, R], BF16, tag="u")
                    for s in range(2):
                        gv = gth[:, JC * s : JC * s + JC, 0:EV].rearrange(
                            "p j (k c) -> p j k c", c=4
                        )
                        wb = (
                            WGH[:, s, JC * ch : JC * ch + JC, :]
                            .unsqueeze(2)
                            .broadcast_to([P, JC, R, 4])
                        )
                        nc.vector.tensor_tensor(m[:, s], gv, wb, OP.mult)
                        nc.vector.tensor_tensor(
                            t2[:, s], m[:, s, :, :, 0:2], m[:, s, :, :, 2:4],
                            OP.add,
                        )
                        nc.vector.tensor_tensor(
                            u[:, s], t2[:, s, :, :, 0], t2[:, s, :, :, 1],
                            OP.add,
                        )
                    pr = cb.tile([P, JC, R], BF16, tag="pr")
                    nc.vector.tensor_tensor(pr[:], u[:, 0], u[:, 1], OP.mult)
                    with nc.allow_low_precision(reason="|y|<~300, bf16 ok"):
                        nc.vector.tensor_reduce(
                            ysb[:, JC * ch : JC * ch + JC], pr[:],
                            mybir.AxisListType.X, OP.add,
                        )

            ysf = pe.tile([P, J], F32)
            nc.vector.tensor_copy(ysf[:], ysb[:])
            nc.sync.dma_start(y_pm[:], ysf[:])
            DEBUG_TILES.update(LS=LS, WGH=WGH, ysb=ysb, fl=fl, wv=wv)

    nc.finalize()
    return nc


def _make_tables(core0, core1, core2, core3):
    """Joint corner-packed bf16 tables, stacked G then H: [2*TE, ES]."""
    c0 = np.asarray(core0, dtype=np.float32)[0]        # [128, 16]
    c1 = np.asarray(core1, dtype=np.float32)           # [16, 128, 16]
    c2 = np.asarray(core2, dtype=np.float32)           # [16, 128, 16]
    c3 = np.asarray(core3, dtype=np.float32)[:, :, 0]  # [16, 128]

    G = np.einsum("ac,cbk->abk", c0, c1)               # [n0, n1, k]
    H = np.einsum("cae,eb->abc", c2, c3)               # [n2, n3, k]

    hi = np.minimum(np.arange(N) + 1, N - 1)

    def pack(T):
        # entry[(a*128+b), k, (dhi,dlo)] = T[a+dhi, b+dlo, k], padded to ES
        cs = np.stack([T, T[:, hi], T[hi], T[hi][:, hi]], axis=-1)
        out = np.zeros((TE, ES), dtype=np.float32)
        out[:, :EV] = cs.reshape(TE, EV)
        return out

    return np.concatenate([pack(G), pack(H)], axis=0).astype(ml_dtypes.bfloat16)


def _prep_inputs(x, core0, core1, core2, core3):
    """Shard x over cores; build the combine-layout copy (x_pm) and the
    wrapped idx-path copy (xq2); attach the shared host-built table."""
    xs = np.ascontiguousarray(np.asarray(x, dtype=np.float32).reshape(NCORES, BS, 4))
    ghd = _make_tables(core0, core1, core2, core3)

    # wrapped idx layout: position i (in chunk ch) = (s*JC + j)*128 + p,
    # global list col C = LCC*ch + (s*JC + j)*8 + p//16, row r = p%16.
    # xq2h[16*(C//LB) + r, C%LB, :] = x[b, (d0, d1)] for point b = jg*128+p,
    # jg = ch*JC + j, dims (0,1) for s=0 and (2,3) for s=1.
    ch_i, s_i, j_i, p_i = np.meshgrid(
        np.arange(NCH), np.arange(2), np.arange(JC), np.arange(P),
        indexing="ij",
    )
    C = LCC * ch_i + (s_i * JC + j_i) * 8 + p_i // 16
    rband = 16 * (C // LB) + p_i % 16
    cband = C % LB
    bidx = (ch_i * JC + j_i) * P + p_i
    d0 = np.where(s_i == 0, 0, 2)
    d1 = d0 + 1

    in_maps = []
    for c in range(NCORES):
        xc_ = xs[c]
        x_pm = np.ascontiguousarray(
            xc_.reshape(J, P, 4).transpose(1, 0, 2)
        )  # [128, 256, 4]
        xq2h = np.empty((64, LB, 2), dtype=np.float32)
        xq2h[rband, cband, 0] = xc_[bidx, d0]
        xq2h[rband, cband, 1] = xc_[bidx, d1]
        in_maps.append({"x_pm": x_pm, "xq2": xq2h, "ghd": ghd})
    return in_maps


def kernel(x, core0, core1, core2, core3):
    global _CACHED
    if _CACHED is None:
        _CACHED = _build_nc()
    nc = _CACHED
    in_maps = _prep_inputs(x, core0, core1, core2, core3)
    res = run_bass_kernel_spmd(nc, in_maps, core_ids=list(range(NCORES)))
    outs = []
    for c in range(NCORES):
        y_pm = res.results[c]["y_pm"]          # [128, 256]
        outs.append(np.ascontiguousarray(np.asarray(y_pm).T).reshape(-1))
    return np.concatenate(outs).astype(np.float32)


# revision 13
# speedup vs baseline: 1.0931x; 1.0931x over previous
"""Trainium2 Bass kernel for nn_ModelConTT_46016279609475 (TT interpolation).

y[b] = v0[b]^T V1[b] V2[b] v3[b], where v_i are linearly-interpolated slices
of tiny TT cores at per-point grid coordinates derived from x[b, :].

Strategy (per NeuronCore, data-parallel over B):
  * Host precomputes two joint corner-packed tables (pure functions of the
    ~1MB cores, so no on-device table build or DRAM writeback):
      G[n0, n1, k] = sum_c core0[n0, c] * core1[c, n1, k]        (u-side)
      H[n2, n3, k] = sum_c core2[k, n2, c] * core3[c, n3]        (v-side)
    packed bf16 rows T[(a*128+b)] = [16 k x 4 corners] + 64 pad = 256B
    (dma_gather's minimum element), stacked G then H in ghd[32768, 128].
  * Per chunk ONE dma_gather fetches both sides (index position
    i = (s*jlen + j)*128 + p lands entry at dst[p, s*jlen + j, :], giving
    a [p, side, j, k, c] output directly).
  * Index lists are built on-device in dma_gather's wrapped layout
    (idx i at [i%16, i//16], replicated to rows 16-31 for queue 0's core
    pair) from a host-rearranged second copy of x (xq2) processed in
    16-row bands at partition bases 0/32/64/96. H-side entries get +16384
    by adding 128.0 to fl_d0 before the *128 combine.
  * Combine on DVE in bf16 (2x mode): m = g * W (corner weights bcast
    over k), pairwise corner adds, u_G * u_H, reduce over k.
  * Software pipelining: chunk ch's idx chain + gather issue one chunk
    ahead of chunk ch-1's combine so the gather DMA stream never waits on
    DVE; small first/last chunks shrink the pipeline fill/drain.
  * Exact-floor trick (f32-safe): t = (xc + 2^23) - 2^23 rounds to
    nearest; g = (t > xc); floor = t - g; frac = xc - floor is exact.

Batch mapping per core: shard b of size 32768; point i lives at
partition i%128, free col i//128 (y_pm[p, j] = y[j*128 + p]).
"""

import numpy as np
import ml_dtypes

import concourse.bass as bass
import concourse.bacc as bacc
import concourse.mybir as mybir
import concourse.tile as tile
from concourse import library_config
from concourse.bass_utils import run_bass_kernel_spmd

F32 = mybir.dt.float32
BF16 = mybir.dt.bfloat16
I16 = mybir.dt.int16
OP = mybir.AluOpType
AF = mybir.ActivationFunctionType

NCORES = 8
B = 262144
BS = B // NCORES          # 32768 points per core
P = 128                   # partitions
J = BS // P               # 256 free cols per partition
CHUNKS = (16, 16, 32, 32, 32, 32, 32, 32, 16, 16)  # j-cols per chunk
LB = J * 32 // 4          # 2048... idx-list cols per 16-row band = LT/4
LT = 2 * BS // 16         # 4096 idx-list cols total
N = 128                   # mode size
R = 16                    # TT rank
TE = N * N                # entries per table
EV = 64                   # useful values per entry: 16 k x 4 corners
ES = 128                  # stored row: EV values + pad to 256B
MAGIC = float(2 ** 23)
SCALE = (N - 1) / 2.0     # 63.5
LOOKAHEAD = 1             # chunks the gather stream runs ahead of combine

assert sum(CHUNKS) == J
assert LT // 4 == 1024
LB = LT // 4

_CACHED = None
DEBUG_TILES = {}


def _chunk_layout():
    """Per chunk: (jstart, jlen, list colstart). Chunks never straddle a
    16-row band (1024 list cols)."""
    out = []
    jstart = 0
    for jlen in CHUNKS:
        cstart = 16 * jstart
        assert cstart // LB == (cstart + 16 * jlen - 1) // LB
        out.append((jstart, jlen, cstart))
        jstart += jlen
    return out


def _build_nc():
    nc = bacc.Bacc("TRN2")

    x_pm = nc.dram_tensor("x_pm", [P, J, 4], F32, kind="ExternalInput")
    xq2 = nc.dram_tensor("xq2", [64, LB, 2], F32, kind="ExternalInput")
    ghd = nc.dram_tensor("ghd", [2 * TE, ES], BF16, kind="ExternalInput")
    y_pm = nc.dram_tensor("y_pm", [P, J], F32, kind="ExternalOutput")

    layout = _chunk_layout()
    nch = len(layout)

    with tile.TileContext(nc) as tc:
        with tc.tile_pool(name="per", bufs=1) as pe:
            nc.gpsimd.load_library(library_config.mlp)

            # idx list in dma_gather wrapped layout; rows 32+ only feed the
            # bounds check, memset once on Pool.
            LS = pe.tile([P, LT], I16)
            nc.gpsimd.memset(LS[:], 0)

            # ---------------- input loads ----------------
            xq2_s = pe.tile([112, LB * 2], F32)
            xq2v = xq2[:].rearrange("p a b -> p (a b)")
            for b in range(4):
                nc.sync.dma_start(
                    xq2_s[32 * b : 32 * b + 16, :], xq2v[16 * b : 16 * b + 16, :]
                )
            x_s = pe.tile([P, J * 4], F32)
            nc.sync.dma_start(x_s[:], x_pm[:].rearrange("p a b -> p (a b)"))

            xc2 = pe.tile([112, LB * 2], F32)
            t12 = pe.tile([112, LB * 2], F32)
            g2 = pe.tile([112, LB * 2], F32)
            fl2 = pe.tile([112, LB * 2], F32)
            WGH = pe.tile([P, 2, J, 4], BF16)
            ysb = pe.tile([P, J], BF16)

            def idx_chain_and_gather(ch, gth):
                jstart, jlen, cstart = layout[ch]
                lcc = 16 * jlen                 # list cols of this chunk
                ni = 2 * jlen * P               # gathered entries
                q = 32 * (cstart // LB)         # partition base of band
                c0 = cstart % LB                # col offset inside band
                fs = slice(2 * c0, 2 * (c0 + lcc))
                rows = slice(q, q + 16)
                nc.scalar.activation(
                    xc2[rows, fs], xq2_s[rows, fs], AF.Copy,
                    bias=SCALE, scale=SCALE,
                )
                nc.scalar.activation(
                    t12[rows, fs], xc2[rows, fs], AF.Copy,
                    bias=MAGIC, scale=1.0,
                )
                nc.vector.scalar_tensor_tensor(
                    g2[rows, fs], t12[rows, fs], -MAGIC, xc2[rows, fs],
                    OP.add, OP.is_gt,
                )
                nc.vector.scalar_tensor_tensor(
                    fl2[rows, fs], t12[rows, fs], -MAGIC, g2[rows, fs],
                    OP.add, OP.subtract,
                )
                f2v = fl2[rows, fs].rearrange("p (c d) -> p c d", d=2)
                # H-side (second half of the chunk's cols): fl_d0 += 128 so
                # idx = (fl_d0+128)*128 + fl_d1 lands in the H table rows.
                nc.vector.tensor_scalar(
                    f2v[:, lcc // 2 :, 0], f2v[:, lcc // 2 :, 0],
                    1.0, 128.0, OP.mult, OP.add,
                )
                nc.vector.scalar_tensor_tensor(
                    LS[rows, cstart : cstart + lcc],
                    f2v[:, :, 0], 128.0, f2v[:, :, 1], OP.mult, OP.add,
                )
                if q != 0:
                    nc.sync.dma_start(
                        LS[0:16, cstart : cstart + lcc],
                        LS[rows, cstart : cstart + lcc],
                    )
                nc.sync.dma_start(
                    LS[16:32, cstart : cstart + lcc],
                    LS[0:16, cstart : cstart + lcc],
                )
                nc.gpsimd.dma_gather(
                    gth[:],
                    ghd[:],
                    LS[:, cstart : cstart + lcc],
                    ni,
                    ni,
                    ES,
                    queue_num=0,
                    single_packet=False,
                )

            def weights_prep():
                xc = pe.tile([P, J * 4], F32)
                nc.scalar.activation(
                    xc[:], x_s[:], AF.Copy, bias=SCALE, scale=SCALE
                )
                t1 = pe.tile([P, J * 4], F32)
                nc.scalar.activation(t1[:], xc[:], AF.Copy, bias=MAGIC, scale=1.0)
                gt = pe.tile([P, J * 4], F32)
                nc.vector.scalar_tensor_tensor(
                    gt[:], t1[:], -MAGIC, xc[:], OP.add, OP.is_gt
                )
                fl = pe.tile([P, J * 4], F32)
                nc.vector.scalar_tensor_tensor(
                    fl[:], t1[:], -MAGIC, gt[:], OP.add, OP.subtract
                )
                wv = pe.tile([P, J * 4], F32)
                nc.vector.tensor_tensor(wv[:], xc[:], fl[:], OP.subtract)
                av = pe.tile([P, J * 4], F32)
                nc.vector.tensor_scalar(av[:], wv[:], -1.0, 1.0, OP.mult, OP.add)
                wvv = wv[:].rearrange("p (j d) -> p j d", d=4)
                avv = av[:].rearrange("p (j d) -> p j d", d=4)
                # corner weights; corner c=(dhi,dlo): c0=a*a, c1=a*w,
                # c2=w*a, c3=w*w over dims (0,1) for G and (2,3) for H
                for s, (d0, d1) in enumerate(((0, 1), (2, 3))):
                    for c, (h0, h1) in enumerate(((avv, avv), (avv, wvv),
                                                  (wvv, avv), (wvv, wvv))):
                        nc.vector.tensor_tensor(
                            WGH[:, s, :, c], h0[:, :, d0], h1[:, :, d1], OP.mult
                        )

            def combine(ch, gth, cb):
                jstart, jlen, _ = layout[ch]
                m = cb.tile([P, 2, jlen, R, 4], BF16, tag="m")
                t2 = cb.tile([P, 2, jlen, R, 2], BF16, tag="t2")
                u = cb.tile([P, 2, jlen, R], BF16, tag="u")
                for s in range(2):
                    gv = gth[:, jlen * s : jlen * s + jlen, 0:EV].rearrange(
                        "p j (k c) -> p j k c", c=4
                    )
                    wb = (
                        WGH[:, s, jstart : jstart + jlen, :]
                        .unsqueeze(2)
                        .broadcast_to([P, jlen, R, 4])
                    )
                    nc.vector.tensor_tensor(m[:, s], gv, wb, OP.mult)
                    nc.vector.tensor_tensor(
                        t2[:, s], m[:, s, :, :, 0:2], m[:, s, :, :, 2:4], OP.add
                    )
                    nc.vector.tensor_tensor(
                        u[:, s], t2[:, s, :, :, 0], t2[:, s, :, :, 1], OP.add
                    )
                pr = cb.tile([P, jlen, R], BF16, tag="pr")
                nc.vector.tensor_tensor(pr[:], u[:, 0], u[:, 1], OP.mult)
                with nc.allow_low_precision(reason="|y|<~300, bf16 ok"):
                    nc.vector.tensor_reduce(
                        ysb[:, jstart : jstart + jlen], pr[:],
                        mybir.AxisListType.X, OP.add,
                    )

            with (
                tc.tile_pool(name="gbuf", bufs=LOOKAHEAD + 2) as gb,
                tc.tile_pool(name="cbuf", bufs=2) as cb,
            ):
                gths = {}
                for it in range(nch + LOOKAHEAD):
                    if it < nch:
                        jlen = layout[it][1]
                        gth = gb.tile([P, 2 * jlen, ES], BF16, tag="gth")
                        idx_chain_and_gather(it, gth)
                        gths[it] = gth
                    if it == 0:
                        weights_prep()
                    if it >= LOOKAHEAD:
                        combine(it - LOOKAHEAD, gths.pop(it - LOOKAHEAD), cb)

            ysf = pe.tile([P, J], F32)
            nc.vector.tensor_copy(ysf[:], ysb[:])
            nc.sync.dma_start(y_pm[:], ysf[:])
            DEBUG_TILES.update(LS=LS, WGH=WGH, ysb=ysb)

    nc.finalize()
    return nc


def _make_tables(core0, core1, core2, core3):
    """Joint corner-packed bf16 tables, stacked G then H: [2*TE, ES]."""
    c0 = np.asarray(core0, dtype=np.float32)[0]        # [128, 16]
    c1 = np.asarray(core1, dtype=np.float32)           # [16, 128, 16]
    c2 = np.asarray(core2, dtype=np.float32)           # [16, 128, 16]
    c3 = np.asarray(core3, dtype=np.float32)[:, :, 0]  # [16, 128]

    G = np.einsum("ac,cbk->abk", c0, c1)               # [n0, n1, k]
    H = np.einsum("cae,eb->abc", c2, c3)               # [n2, n3, k]

    hi = np.minimum(np.arange(N) + 1, N - 1)

    def pack(T):
        # entry[(a*128+b), k, (dhi,dlo)] = T[a+dhi, b+dlo, k], padded to ES
        cs = np.stack([T, T[:, hi], T[hi], T[hi][:, hi]], axis=-1)
        out = np.zeros((TE, ES), dtype=np.float32)
        out[:, :EV] = cs.reshape(TE, EV)
        return out

    return np.concatenate([pack(G), pack(H)], axis=0).astype(ml_dtypes.bfloat16)


def _prep_inputs(x, core0, core1, core2, core3):
    """Shard x over cores; build the combine-layout copy (x_pm) and the
    wrapped idx-path copy (xq2); attach the shared host-built table."""
    xs = np.ascontiguousarray(np.asarray(x, dtype=np.float32).reshape(NCORES, BS, 4))
    ghd = _make_tables(core0, core1, core2, core3)

    # wrapped idx layout: within chunk ch, position i = (s*jlen + j)*128 + p,
    # global list col C = cstart + (s*jlen + j)*8 + p//16, row r = p%16.
    # xq2h[16*(C//LB) + r, C%LB, :] = x[b, (d0, d1)] for b = (jstart+j)*128+p,
    # dims (0,1) for s=0 and (2,3) for s=1.
    Cl, rl, bl, d0l = [], [], [], []
    jstart = 0
    for jlen in CHUNKS:
        cstart = 16 * jstart
        s_i, j_i, p_i = np.meshgrid(
            np.arange(2), np.arange(jlen), np.arange(P), indexing="ij"
        )
        Cl.append(cstart + (s_i * jlen + j_i) * 8 + p_i // 16)
        rl.append(p_i % 16)
        bl.append((jstart + j_i) * P + p_i)
        d0l.append(np.where(s_i == 0, 0, 2))
        jstart += jlen
    C = np.concatenate([a.ravel() for a in Cl])
    rr = np.concatenate([a.ravel() for a in rl])
    bb = np.concatenate([a.ravel() for a in bl])
    dd0 = np.concatenate([a.ravel() for a in d0l])
    rband = 16 * (C // LB) + rr
    cband = C % LB

    in_maps = []
    for c in range(NCORES):
        xc_ = xs[c]
        x_pm = np.ascontiguousarray(
            xc_.reshape(J, P, 4).transpose(1, 0, 2)
        )  # [128, 256, 4]
        xq2h = np.empty((64, LB, 2), dtype=np.float32)
        xq2h[rband, cband, 0] = xc_[bb, dd0]
        xq2h[rband, cband, 1] = xc_[bb, dd0 + 1]
        in_maps.append({"x_pm": x_pm, "xq2": xq2h, "ghd": ghd})
    return in_maps


def kernel(x, core0, core1, core2, core3):
    global _CACHED
    if _CACHED is None:
        _CACHED = _build_nc()
    nc = _CACHED
    in_maps = _prep_inputs(x, core0, core1, core2, core3)
    res = run_bass_kernel_spmd(nc, in_maps, core_ids=list(range(NCORES)))
    outs = []
    for c in range(NCORES):
        y_pm = res.results[c]["y_pm"]          # [128, 256]
        outs.append(np.ascontiguousarray(np.asarray(y_pm).T).reshape(-1))
    return np.concatenate(outs).astype(np.float32)


# revision 18
# speedup vs baseline: 1.2691x; 1.1610x over previous
"""Trainium2 Bass kernel for nn_ModelConTT_46016279609475 (TT interpolation).

y[b] = v0[b]^T V1[b] V2[b] v3[b], where v_i are linearly-interpolated slices
of tiny TT cores at per-point grid coordinates derived from x[b, :].

Strategy (per NeuronCore, data-parallel over B):
  * Host precomputes two joint corner-packed tables (pure functions of the
    ~1MB cores, so no on-device table build or DRAM writeback):
      G[n0, n1, k] = sum_c core0[n0, c] * core1[c, n1, k]        (u-side)
      H[n2, n3, k] = sum_c core2[k, n2, c] * core3[c, n3]        (v-side)
    packed bf16 rows T[(a*128+b)] = [16 k x 4 corners] + 64 pad = 256B
    (dma_gather's minimum element), stacked G then H in ghd[32768, 128].
  * Per chunk ONE dma_gather fetches both sides (index position
    i = (s*jlen + j)*128 + p lands entry at dst[p, s*jlen + j, :], giving
    a [p, side, j, k, c] output directly).
  * Index lists are built on-device in dma_gather's wrapped layout
    (idx i at [i%16, i//16], replicated to rows 16-31 for queue 0's core
    pair) from a host-rearranged second copy of x (xq2) processed in
    16-row bands at partition bases 0/32/64/96. H-side entries get +16384
    by adding 128.0 to fl_d0 before the *128 combine.
  * Combine on DVE in bf16 (2x mode): m = g * W (corner weights bcast
    over k), pairwise corner adds, u_G * u_H, reduce over k.
  * Software pipelining: chunk ch's idx chain + gather issue one chunk
    ahead of chunk ch-1's combine so the gather DMA stream never waits on
    DVE; small first/last chunks shrink the pipeline fill/drain.
  * Exact-floor trick (f32-safe): t = (xc + 2^23) - 2^23 rounds to
    nearest; g = (t > xc); floor = t - g; frac = xc - floor is exact.

Batch mapping per core: shard b of size 32768; point i lives at
partition i%128, free col i//128 (y_pm[p, j] = y[j*128 + p]).
"""

import numpy as np
import ml_dtypes

import concourse.bass as bass
import concourse.bacc as bacc
import concourse.mybir as mybir
import concourse.tile as tile
from concourse import library_config
from concourse.bass_utils import run_bass_kernel_spmd

F32 = mybir.dt.float32
BF16 = mybir.dt.bfloat16
I16 = mybir.dt.int16
OP = mybir.AluOpType
AF = mybir.ActivationFunctionType

NCORES = 8
B = 262144
BS = B // NCORES          # 32768 points per core
P = 128                   # partitions
J = BS // P               # 256 free cols per partition
CHUNKS = (16, 16, 32, 32, 32, 32, 32, 32, 16, 16)  # j-cols per chunk
LB = J * 32 // 4          # 2048... idx-list cols per 16-row band = LT/4
LT = 2 * BS // 16         # 4096 idx-list cols total
N = 128                   # mode size
R = 16                    # TT rank
TE = N * N                # entries per table
EV = 64                   # useful values per entry: 16 k x 4 corners
ES = 128                  # stored row: EV values + pad to 256B
MAGIC = float(2 ** 23)
SCALE = (N - 1) / 2.0     # 63.5
LOOKAHEAD = 1             # chunks the gather stream runs ahead of combine
CHAIN_AHEAD = 3           # chunks the idx chains run ahead of gathers

assert sum(CHUNKS) == J
assert LT // 4 == 1024
LB = LT // 4

_CACHED = None
DEBUG_TILES = {}


def _chunk_layout():
    """Per chunk: (jstart, jlen, list colstart). Chunks never straddle a
    16-row band (1024 list cols)."""
    out = []
    jstart = 0
    for jlen in CHUNKS:
        cstart = 16 * jstart
        assert cstart // LB == (cstart + 16 * jlen - 1) // LB
        out.append((jstart, jlen, cstart))
        jstart += jlen
    return out


def _build_nc():
    nc = bacc.Bacc("TRN2")

    x_pm = nc.dram_tensor("x_pm", [P, J, 4], F32, kind="ExternalInput")
    xq2 = nc.dram_tensor("xq2", [64, LB, 2], F32, kind="ExternalInput")
    ghd = nc.dram_tensor("ghd", [2 * TE, ES], BF16, kind="ExternalInput")
    y_pm = nc.dram_tensor("y_pm", [P, J], F32, kind="ExternalOutput")

    layout = _chunk_layout()
    nch = len(layout)

    with tile.TileContext(nc) as tc:
        with tc.tile_pool(name="per", bufs=1) as pe:
            nc.gpsimd.load_library(library_config.mlp)

            # idx list in dma_gather wrapped layout; rows 32+ only feed the
            # bounds check, memset once on Pool.
            LS = pe.tile([P, LT], I16)
            nc.gpsimd.memset(LS[:], 0)

            # ---------------- input loads ----------------
            xq2_s = pe.tile([112, LB * 2], F32)
            xq2v = xq2[:].rearrange("p a b -> p (a b)")
            for b in range(4):
                nc.sync.dma_start(
                    xq2_s[32 * b : 32 * b + 16, :], xq2v[16 * b : 16 * b + 16, :]
                )
            x_s = pe.tile([P, J * 4], F32)
            nc.sync.dma_start(x_s[:], x_pm[:].rearrange("p a b -> p (a b)"))

            xc2 = pe.tile([112, LB * 2], F32)
            t12 = pe.tile([112, LB * 2], F32)
            g2 = pe.tile([112, LB * 2], F32)
            fl2 = pe.tile([112, LB * 2], F32)
            WGH = pe.tile([P, 2, J, 4], BF16)

            def idx_chain(ch):
                jstart, jlen, cstart = layout[ch]
                lcc = 16 * jlen                 # list cols of this chunk
                q = 32 * (cstart // LB)         # partition base of band
                c0 = cstart % LB                # col offset inside band
                fs = slice(2 * c0, 2 * (c0 + lcc))
                rows = slice(q, q + 16)
                # chunk 0 is latency-critical: keep its chain on one engine
                # to skip two cross-engine semaphore hops
                seng = nc.vector if ch == 0 else nc.scalar
                if ch == 0:
                    nc.vector.tensor_scalar(
                        xc2[rows, fs], xq2_s[rows, fs], SCALE, SCALE,
                        OP.mult, OP.add,
                    )
                    nc.vector.tensor_scalar(
                        t12[rows, fs], xc2[rows, fs], 1.0, MAGIC,
                        OP.mult, OP.add,
                    )
                else:
                    nc.scalar.activation(
                        xc2[rows, fs], xq2_s[rows, fs], AF.Copy,
                        bias=SCALE, scale=SCALE,
                    )
                    nc.scalar.activation(
                        t12[rows, fs], xc2[rows, fs], AF.Copy,
                        bias=MAGIC, scale=1.0,
                    )
                nc.vector.scalar_tensor_tensor(
                    g2[rows, fs], t12[rows, fs], -MAGIC, xc2[rows, fs],
                    OP.add, OP.is_gt,
                )
                nc.vector.scalar_tensor_tensor(
                    fl2[rows, fs], t12[rows, fs], -MAGIC, g2[rows, fs],
                    OP.add, OP.subtract,
                )
                f2v = fl2[rows, fs].rearrange("p (c d) -> p c d", d=2)
                # H-side (second half of the chunk's cols): fl_d0 += 128 so
                # idx = (fl_d0+128)*128 + fl_d1 lands in the H table rows.
                nc.vector.tensor_scalar(
                    f2v[:, lcc // 2 :, 0], f2v[:, lcc // 2 :, 0],
                    1.0, 128.0, OP.mult, OP.add,
                )
                nc.vector.scalar_tensor_tensor(
                    LS[rows, cstart : cstart + lcc],
                    f2v[:, :, 0], 128.0, f2v[:, :, 1], OP.mult, OP.add,
                )
                if q != 0:
                    nc.sync.dma_start(
                        LS[0:16, cstart : cstart + lcc],
                        LS[rows, cstart : cstart + lcc],
                    )
                nc.sync.dma_start(
                    LS[16:32, cstart : cstart + lcc],
                    LS[0:16, cstart : cstart + lcc],
                )

            def gather(ch, gth):
                _, jlen, cstart = layout[ch]
                lcc = 16 * jlen
                ni = 2 * jlen * P
                nc.gpsimd.dma_gather(
                    gth[:],
                    ghd[:],
                    LS[:, cstart : cstart + lcc],
                    ni,
                    ni,
                    ES,
                    queue_num=0,
                    single_packet=False,
                )

            def weights_prep():
                xc = pe.tile([P, J * 4], F32)
                nc.scalar.activation(
                    xc[:], x_s[:], AF.Copy, bias=SCALE, scale=SCALE
                )
                t1 = pe.tile([P, J * 4], F32)
                nc.scalar.activation(t1[:], xc[:], AF.Copy, bias=MAGIC, scale=1.0)
                gt = pe.tile([P, J * 4], F32)
                nc.vector.scalar_tensor_tensor(
                    gt[:], t1[:], -MAGIC, xc[:], OP.add, OP.is_gt
                )
                fl = pe.tile([P, J * 4], F32)
                nc.vector.scalar_tensor_tensor(
                    fl[:], t1[:], -MAGIC, gt[:], OP.add, OP.subtract
                )
                wv = pe.tile([P, J * 4], F32)
                nc.vector.tensor_tensor(wv[:], xc[:], fl[:], OP.subtract)
                av = pe.tile([P, J * 4], F32)
                nc.vector.tensor_scalar(av[:], wv[:], -1.0, 1.0, OP.mult, OP.add)
                wvv = wv[:].rearrange("p (j d) -> p j d", d=4)
                avv = av[:].rearrange("p (j d) -> p j d", d=4)
                # corner weights; corner c=(dhi,dlo): c0=a*a, c1=a*w,
                # c2=w*a, c3=w*w over dims (0,1) for G and (2,3) for H
                for s, (d0, d1) in enumerate(((0, 1), (2, 3))):
                    for c, (h0, h1) in enumerate(((avv, avv), (avv, wvv),
                                                  (wvv, avv), (wvv, wvv))):
                        nc.vector.tensor_tensor(
                            WGH[:, s, :, c], h0[:, :, d0], h1[:, :, d1], OP.mult
                        )

            def combine(ch, gth, cb):
                jstart, jlen, _ = layout[ch]
                m = cb.tile([P, 2, jlen, R, 4], BF16, tag="m")
                t2 = cb.tile([P, 2, jlen, R, 2], BF16, tag="t2")
                u = cb.tile([P, 2, jlen, R], BF16, tag="u")
                for s in range(2):
                    gv = gth[:, jlen * s : jlen * s + jlen, 0:EV].rearrange(
                        "p j (k c) -> p j k c", c=4
                    )
                    wb = (
                        WGH[:, s, jstart : jstart + jlen, :]
                        .unsqueeze(2)
                        .broadcast_to([P, jlen, R, 4])
                    )
                    nc.vector.tensor_tensor(m[:, s], gv, wb, OP.mult)
                    nc.vector.tensor_tensor(
                        t2[:, s], m[:, s, :, :, 0:2], m[:, s, :, :, 2:4], OP.add
                    )
                    nc.vector.tensor_tensor(
                        u[:, s], t2[:, s, :, :, 0], t2[:, s, :, :, 1], OP.add
                    )
                pr = cb.tile([P, jlen, R], BF16, tag="pr")
                nc.vector.tensor_tensor(pr[:], u[:, 0], u[:, 1], OP.mult)
                nc.vector.tensor_reduce(
                    ysf[:, jstart : jstart + jlen], pr[:],
                    mybir.AxisListType.X, OP.add,
                )

            ysf = pe.tile([P, J], F32)

            with (
                tc.tile_pool(name="gbuf", bufs=3) as gb,
                tc.tile_pool(name="cbuf", bufs=2) as cb,
            ):
                gths = {}
                for ch in range(CHAIN_AHEAD):
                    idx_chain(ch)
                for ch in range(nch):
                    jlen = layout[ch][1]
                    gth = gb.tile([P, 2 * jlen, ES], BF16, tag="gth")
                    gather(ch, gth)
                    gths[ch] = gth
                    if ch == 0:
                        weights_prep()
                    if ch + CHAIN_AHEAD < nch:
                        idx_chain(ch + CHAIN_AHEAD)
                    if ch >= LOOKAHEAD:
                        combine(ch - LOOKAHEAD, gths.pop(ch - LOOKAHEAD), cb)
                for ch in range(nch - LOOKAHEAD, nch):
                    combine(ch, gths.pop(ch), cb)

            nc.sync.dma_start(y_pm[:], ysf[:])
            DEBUG_TILES.update(LS=LS, WGH=WGH, ysf=ysf)

    nc.finalize()
    return nc


def _make_tables(core0, core1, core2, core3):
    """Joint corner-packed bf16 tables, stacked G then H: [2*TE, ES]."""
    c0 = np.asarray(core0, dtype=np.float32)[0]        # [128, 16]
    c1 = np.asarray(core1, dtype=np.float32)           # [16, 128, 16]
    c2 = np.asarray(core2, dtype=np.float32)           # [16, 128, 16]
    c3 = np.asarray(core3, dtype=np.float32)[:, :, 0]  # [16, 128]

    G = np.einsum("ac,cbk->abk", c0, c1)               # [n0, n1, k]
    H = np.einsum("cae,eb->abc", c2, c3)               # [n2, n3, k]

    hi = np.minimum(np.arange(N) + 1, N - 1)

    def pack(T):
        # entry[(a*128+b), k, (dhi,dlo)] = T[a+dhi, b+dlo, k], padded to ES
        cs = np.stack([T, T[:, hi], T[hi], T[hi][:, hi]], axis=-1)
        out = np.zeros((TE, ES), dtype=np.float32)
        out[:, :EV] = cs.reshape(TE, EV)
        return out

    return np.concatenate([pack(G), pack(H)], axis=0).astype(ml_dtypes.bfloat16)


def _prep_inputs(x, core0, core1, core2, core3):
    """Shard x over cores; build the combine-layout copy (x_pm) and the
    wrapped idx-path copy (xq2); attach the shared host-built table."""
    xs = np.ascontiguousarray(np.asarray(x, dtype=np.float32).reshape(NCORES, BS, 4))
    ghd = _make_tables(core0, core1, core2, core3)

    # wrapped idx layout: within chunk ch, position i = (s*jlen + j)*128 + p,
    # global list col C = cstart + (s*jlen + j)*8 + p//16, row r = p%16.
    # xq2h[16*(C//LB) + r, C%LB, :] = x[b, (d0, d1)] for b = (jstart+j)*128+p,
    # dims (0,1) for s=0 and (2,3) for s=1.
    Cl, rl, bl, d0l = [], [], [], []
    jstart = 0
    for jlen in CHUNKS:
        cstart = 16 * jstart
        s_i, j_i, p_i = np.meshgrid(
            np.arange(2), np.arange(jlen), np.arange(P), indexing="ij"
        )
        Cl.append(cstart + (s_i * jlen + j_i) * 8 + p_i // 16)
        rl.append(p_i % 16)
        bl.append((jstart + j_i) * P + p_i)
        d0l.append(np.where(s_i == 0, 0, 2))
        jstart += jlen
    C = np.concatenate([a.ravel() for a in Cl])
    rr = np.concatenate([a.ravel() for a in rl])
    bb = np.concatenate([a.ravel() for a in bl])
    dd0 = np.concatenate([a.ravel() for a in d0l])
    rband = 16 * (C // LB) + rr
    cband = C % LB

    in_maps = []
    for c in range(NCORES):
        xc_ = xs[c]
        x_pm = np.ascontiguousarray(
            xc_.reshape(J, P, 4).transpose(1, 0, 2)
        )  # [128, 256, 4]
        xq2h = np.empty((64, LB, 2), dtype=np.float32)
        xq2h[rband, cband, 0] = xc_[bb, dd0]
        xq2h[rband, cband, 1] = xc_[bb, dd0 + 1]
        in_maps.append({"x_pm": x_pm, "xq2": xq2h, "ghd": ghd})
    return in_maps


def kernel(x, core0, core1, core2, core3):
    global _CACHED
    if _CACHED is None:
        _CACHED = _build_nc()
    nc = _CACHED
    in_maps = _prep_inputs(x, core0, core1, core2, core3)
    res = run_bass_kernel_spmd(nc, in_maps, core_ids=list(range(NCORES)))
    outs = []
    for c in range(NCORES):
        y_pm = res.results[c]["y_pm"]          # [128, 256]
        outs.append(np.ascontiguousarray(np.asarray(y_pm).T).reshape(-1))
    return np.concatenate(outs).astype(np.float32)
